# revision 53
# baseline (speedup 1.0000x reference)
"""Linformer attention TRN2 Bass kernel (bf16 matmul path).

Problem: nn_LinformerAttention (B=4, L=4096, D=1024, NH=16, DH=64, k=128).

Sharding: 8 cores = batch(4) x head-group(2). Core c handles batch c%4 and
heads (c//4)*8 .. +8, producing out[b, :, hg*512:(hg+1)*512]. Slices are
disjoint -> no collectives; host reassembles.

Device algorithm per core (matmul operands bf16, PSUM/accum fp32):
  phase 1, streamed over 8 l-chunks of 512:
    - K = x @ Wk.T + bk, V likewise (PSUM accum over 8 d-subtiles of 128,
      K/V matmuls interleaved per-dc), cast to bf16 in SBUF
    - Q.T = Wq @ x.T + bq (1/sqrt(dh) folded into Wq/bq on host), kept
      RESIDENT in SBUF as bf16 (no DRAM spill); Q matmuls interleaved 1:1
      with the Linformer psC matmuls so the small matmuls' LDWEIGHTS
      prefetch under the 512-row Q matmuls
    - KVp pair tiles [kk, {K,V}, dh-pair] += E_h-chunk.T @ [K_h | V_h],
      accumulated in fp32 SBUF via one DVE add per head-pair
  phase 2 (phase-1 PSUM pools released; psD pool [128, 2, 512] x2 + psX
  pool x4), one flat (lc, head-pair) stream with a 1-step psX stagger:
    - KpT pair tiles [128, kk]: heads 2j/2j+1 at partitions 0:64/64:128
      (single PE transpose per pair); Vp_aug[h] = [Vp[h] | ones] bf16
    - dotT pair [kk, 2, l-chunk] = two matmuls; exp via ONE ACT op per
      pair (bf16 out, no max-subtraction: logits small by construction)
    - Xo_aug wide tile [l, 4, 65] = 4 matmuls vs Vp_aug; col 64 = denom;
      ONE strided reciprocal + ONE wide multiply per head (DVE op count
      is the phase-2 bottleneck), output bf16
Startup: x0/wk on the sync HWDGE queue, wv/e0/biases/wq on the Activation
HWDGE queue in parallel. All DRAM tensors pre-blocked on host so every DMA
descriptor is a contiguous 8KB per-partition run.

Host prep (numpy, outside HW-timed region): per-partition blocking of
x/W/E/biases, bf16 casts, and the final [li, lc, lt, J] -> [L, J]
output unblock + fp32 cast.
"""

import sys

sys.path.insert(0, "/opt/trn_rl_repo")

import math
from contextlib import ExitStack

import numpy as np
from ml_dtypes import bfloat16 as np_bf16

import json

import concourse.bass as bass
import concourse.bass2jax as bass2jax
import concourse.mybir as mybir
import concourse.tile as tile
from concourse.bass_utils import compile_bir_kernel as _orig_compile_bir_kernel
from concourse.bass_utils import run_bass_kernel_spmd
from concourse.masks import make_identity


def _split_multiwaits(bir_json_bytes):
    """This container's walrus encodes at most ONE sync wait per engine
    instruction ("Too many sync wait commands" otherwise), while Tile emits
    multi-wait instructions. Hoist extra waits onto single-wait
    EventSemaphore carrier instructions placed just before, on the same
    engine queue — semantically identical stalling."""
    bj = json.loads(bir_json_bytes)
    for fn in bj["functions"]:
        for blk in fn["blocks"]:
            out = []
            for inst in blk["instructions"]:
                si = inst.get("sync_info")
                waits = (si or {}).get("on_wait") or []
                if si and len(waits) > 1:
                    for wi, w in enumerate(waits[:-1]):
                        out.append(
                            {
                                "debug": inst.get("debug", 0),
                                "engine": inst.get("engine"),
                                "ins": [],
                                "outs": [],
                                "name": inst["name"] + "-w%d" % wi,
                                "opcode": "EventSemaphore",
                                "sync_info": {"on_update": [], "on_wait": [w]},
                            }
                        )
                    si["on_wait"] = [waits[-1]]
                out.append(inst)
            blk["instructions"] = out
    return json.dumps(bj).encode()


def _patched_compile_bir_kernel(bir_json, tmpdir, neff_name="file.neff"):
    return _orig_compile_bir_kernel(_split_multiwaits(bir_json), tmpdir, neff_name)


bass2jax.compile_bir_kernel = _patched_compile_bir_kernel

B, L, D = 4, 4096, 1024
NH, DH, KK = 16, 64, 128
NCORES = 8
HGS = 2  # head groups
H = NH // HGS  # 8 local heads per core
J = H * DH  # 512 output columns per core
P = 128
LCH = 512  # l-chunk
NLC = L // LCH  # 8
DC = D // P  # 8 contraction subtiles
JT = J // P  # 4
LT4 = LCH // P  # 4 l-tiles per chunk
F32 = mybir.dt.float32
BF16 = mybir.dt.bfloat16

TRACE = False  # test.py sets True to collect a profile
LAST_RESULTS = None  # BassKernelResults of the last kernel() call

_PROGRAM = None


def _build_program():
    nc = bass.Bass()
    # all tensors pre-blocked on host so every DMA descriptor is one
    # contiguous 8KB per-partition run (1KB descriptors cap a DMA queue at
    # ~100GB/s via descriptor-issue rate)
    xT = nc.declare_dram_parameter("xT", [P, NLC, DC, LCH], BF16, isOutput=False)
    wqT = nc.declare_dram_parameter("wqT", [P, DC, J], BF16, isOutput=False)
    wkT = nc.declare_dram_parameter("wkT", [P, DC, J], BF16, isOutput=False)
    wvT = nc.declare_dram_parameter("wvT", [P, DC, J], BF16, isOutput=False)
    bqT = nc.declare_dram_parameter("bqT", [P, JT], F32, isOutput=False)
    bkB = nc.declare_dram_parameter("bkB", [P, J], F32, isOutput=False)
    bvB = nc.declare_dram_parameter("bvB", [P, J], F32, isOutput=False)
    eT = nc.declare_dram_parameter("eT", [NLC, P, H, LT4, KK], BF16, isOutput=False)
    out = nc.declare_dram_parameter("out", [P, NLC, LT4, J], BF16, isOutput=True)

    add = mybir.AluOpType.add
    mult = mybir.AluOpType.mult

    with tile.TileContext(nc) as tc:
        with ExitStack() as ctx:
            const = ctx.enter_context(tc.tile_pool(name="const", bufs=1))
            xpool = ctx.enter_context(tc.tile_pool(name="x", bufs=2))
            kvpool = ctx.enter_context(tc.tile_pool(name="kv", bufs=4))
            epool = ctx.enter_context(tc.tile_pool(name="e", bufs=2))
            exppool = ctx.enter_context(tc.tile_pool(name="ex", bufs=3))
            outpool = ctx.enter_context(tc.tile_pool(name="ot", bufs=3))
            recpool = ctx.enter_context(tc.tile_pool(name="rc", bufs=8))


            # ---- constants resident in SBUF (K/V weights first: first matmuls
            # need only wk/wv + the first x chunk; wq is DMA'd after the first
            # chunk's x/e so it doesn't delay them in the queue)
            wq_sb = const.tile([P, DC, J], BF16, tag="wq")
            wk_sb = const.tile([P, DC, J], BF16, tag="wk")
            wv_sb = const.tile([P, DC, J], BF16, tag="wv")
            bqT_sb = const.tile([P, JT], F32, tag="bqT")
            bkB_sb = const.tile([P, J], F32, tag="bkB")
            bvB_sb = const.tile([P, J], F32, tag="bvB")
            ident = const.tile([P, P], F32, tag="ident")
            make_identity(nc, ident[:])

            # Q kept resident in SBUF, layout [j%128, lc, j//128, l%512]
            qt_all = const.tile([P, NLC, JT, LCH], BF16, tag="qt")

            # per head-pair accum: [kk, {K,V}, dh-of-head-2j | dh-of-head-2j+1]
            kvp2 = [const.tile([P, 2, 2 * DH], F32, tag=f"kvp{j}", name=f"kvp{j}") for j in range(JT)]
            # per head-pair: heads 2j, 2j+1 at partitions 0:64 / 64:128
            kpT2 = [const.tile([P, KK], BF16, tag=f"kpT{j}", name=f"kpT{j}") for j in range(JT)]
            vpa = [const.tile([P, DH + 1], BF16, tag=f"vpa{h}", name=f"vpa{h}") for h in range(H)]



            # ---- phase 1: projections + Linformer K/V reduction
            with ExitStack() as ctx1:
                psA = ctx1.enter_context(tc.tile_pool(name="psA", bufs=4, space="PSUM"))
                psB = ctx1.enter_context(tc.tile_pool(name="psB", bufs=4, space="PSUM"))

                for lc in range(NLC):
                    x_sb = xpool.tile([P, DC, LCH], BF16, tag="x")
                    e_sb = epool.tile([P, H, LT4, KK], BF16, tag="e")
                    if lc == 0:
                        # startup: stream x0/wk in interleaved dc-quarters on
                        # the sync queue so the first K matmuls start after
                        # ~512KB instead of after both full tensors; wv and
                        # the rest ride the (later-starting) Activation queue
                        for q in range(4):
                            ds = slice(2 * q, 2 * q + 2)
                            nc.sync.dma_start(x_sb[:, ds, :], xT[:, lc, ds, :])
                            nc.sync.dma_start(wk_sb[:, ds, :], wkT[:, ds, :])
                        nc.scalar.dma_start(wv_sb[:], wvT[:, :, :])
                        nc.scalar.dma_start(e_sb[:], eT[lc])
                        nc.scalar.dma_start(bkB_sb[:], bkB[:, :])
                        nc.scalar.dma_start(bvB_sb[:], bvB[:, :])
                        nc.scalar.dma_start(bqT_sb[:], bqT[:, :])
                        nc.scalar.dma_start(wq_sb[:], wqT[:, :, :])
                    else:
                        nc.sync.dma_start(x_sb[:], xT[:, lc])
                        nc.sync.dma_start(e_sb[:], eT[lc])
                    kv_tiles = []
                    for lt in range(LT4):
                        psK = psA.tile([P, LCH], F32, tag="big")
                        psV = psA.tile([P, LCH], F32, tag="big")
                        if lc == 0 and lt == 0:
                            # all-K first so compute starts before wv lands
                            for dc in range(DC):
                                nc.tensor.matmul(
                                    psK[:], x_sb[:, dc, lt * P : (lt + 1) * P],
                                    wk_sb[:, dc, :],
                                    start=(dc == 0), stop=(dc == DC - 1),
                                )
                            for dc in range(DC):
                                nc.tensor.matmul(
                                    psV[:], x_sb[:, dc, lt * P : (lt + 1) * P],
                                    wv_sb[:, dc, :],
                                    start=(dc == 0), stop=(dc == DC - 1),
                                )
                        else:
                            for dc in range(DC):
                                xst = x_sb[:, dc, lt * P : (lt + 1) * P]
                                nc.tensor.matmul(
                                    psK[:], xst,
                                    wk_sb[:, dc, :],
                                    start=(dc == 0), stop=(dc == DC - 1),
                                )
                                nc.tensor.matmul(
                                    psV[:], xst,
                                    wv_sb[:, dc, :],
                                    start=(dc == 0), stop=(dc == DC - 1),
                                )
                        kv_sb = kvpool.tile([P, 2, LCH], BF16, tag="kv")
                        nc.any.tensor_tensor(kv_sb[:, 0, :], psK[:], bkB_sb[:], add)
                        nc.any.tensor_tensor(kv_sb[:, 1, :], psV[:], bvB_sb[:], add)
                        kv_tiles.append(kv_sb)
                    for jt in range(JT):
                        psQ = psA.tile([P, LCH], F32, tag="big")
                        psC = psB.tile([P, 2, 2 * DH], F32, tag="small")
                        # Q (512-row) and Linformer psC (128-row) matmuls
                        # interleaved 1:1 so every small matmul's LDWEIGHTS
                        # prefetches under the preceding big matmul
                        for dc in range(DC):
                            nc.tensor.matmul(
                                psQ[:], wq_sb[:, dc, jt * P : (jt + 1) * P],
                                x_sb[:, dc, :],
                                start=(dc == 0), stop=(dc == DC - 1),
                            )
                            hp = dc // LT4  # head parity within the pair
                            h = 2 * jt + hp
                            lt = dc % LT4
                            nc.tensor.matmul(
                                psC[:, :, hp * DH : (hp + 1) * DH],
                                e_sb[:, h, lt, :],
                                kv_tiles[lt][:, :, h * DH : (h + 1) * DH],
                                start=(lt == 0), stop=(lt == LT4 - 1),
                            )
                        nc.any.tensor_scalar(
                            qt_all[:, lc, jt, :], psQ[:], bqT_sb[:, jt : jt + 1], None, add
                        )
                        if lc == 0:
                            nc.any.tensor_copy(kvp2[jt][:], psC[:])
                        else:
                            nc.any.tensor_tensor(kvp2[jt][:], kvp2[jt][:], psC[:], add)
                        if lc == NLC - 1:
                            # kvp2[jt] is final — transpose Kp and prep
                            # Vp-aug for this pair now, overlapping the
                            # remaining jt iterations' matmuls
                            psT = psB.tile([P, KK], F32, tag="small")
                            nc.tensor.transpose(psT[:], kvp2[jt][:, 0, :], ident[:])
                            nc.any.tensor_copy(kpT2[jt][:], psT[:])
                            for hp in range(2):
                                h = 2 * jt + hp
                                nc.any.tensor_copy(
                                    vpa[h][:, 0:DH],
                                    kvp2[jt][:, 1, hp * DH : (hp + 1) * DH],
                                )
                                nc.any.memset(vpa[h][:, DH : DH + 1], 1.0)

            # ---- phase 2: attention (psA/psB released; one wide PSUM pool
            # shared by the dot tiles and the psX tiles — 4 bufs x 2 banks)
            with ExitStack() as ctx2:
                psDp = ctx2.enter_context(tc.tile_pool(name="psD", bufs=2, space="PSUM"))
                psXp = ctx2.enter_context(tc.tile_pool(name="psX", bufs=4, space="PSUM"))

                def psx_group(h, ex2, hp, ot):
                    # all 4 l-tiles of this head into one wide PSUM tile, then
                    # a single strided reciprocal + a single wide multiply
                    psXw = psXp.tile([P, LT4, DH + 1], F32, tag="xw")
                    for lt in range(LT4):
                        nc.tensor.matmul(
                            psXw[:, lt, :], ex2[:, hp, lt * P : (lt + 1) * P], vpa[h][:],
                            start=True, stop=True,
                        )
                    rc = recpool.tile([P, LT4, 1], F32, tag="rc")
                    nc.vector.reciprocal(rc[:], psXw[:, :, DH : DH + 1])
                    nc.any.tensor_tensor(
                        ot[:, :, h * DH : (h + 1) * DH],
                        psXw[:, :, 0:DH],
                        rc[:].to_broadcast([P, LT4, DH]),
                        mult,
                    )

                # one flat stream of (lc, pair) steps with a 1-step stagger
                # carried across lc boundaries: pair s-1's psX smalls are
                # emitted after pair s's dots so LDWEIGHTS prefetch hides
                steps = [(lc, j) for lc in range(NLC) for j in range(JT)]
                prev = None
                ot_cur = None

                def flush_prev(prev):
                    plc, pj, pex, pot = prev
                    for hp in range(2):
                        psx_group(2 * pj + hp, pex, hp, pot)
                    if pj == JT - 1:
                        nc.sync.dma_start(out[:, plc], pot[:])

                for lc, j in steps:
                    if j == 0:
                        ot_cur = outpool.tile([P, LT4, J], BF16, tag="ot")
                    psD2 = psDp.tile([P, 2, LCH], F32, tag="d2")
                    for hp in range(2):
                        par = hp * DH
                        nc.tensor.matmul(
                            psD2[:, hp, :],
                            kpT2[j][par : par + DH, :],
                            qt_all[par : par + DH, lc, j, :],
                            start=True, stop=True,
                        )
                    ex2 = exppool.tile([P, 2, LCH], BF16, tag="ex")
                    nc.scalar.activation(
                        ex2[:], psD2[:], mybir.ActivationFunctionType.Exp
                    )
                    if prev is not None:
                        flush_prev(prev)
                    prev = (lc, j, ex2, ot_cur)
                flush_prev(prev)

    return nc


def _get_program():
    global _PROGRAM
    if _PROGRAM is None:
        _PROGRAM = _build_program()
    return _PROGRAM


def kernel(x, Wq, bq, Wk, bk, Wv, bv, E):
    global LAST_RESULTS
    x = np.ascontiguousarray(np.asarray(x, dtype=np.float32))
    Wq = np.asarray(Wq, dtype=np.float32)
    bq = np.asarray(bq, dtype=np.float32)
    Wk = np.asarray(Wk, dtype=np.float32)
    bk = np.asarray(bk, dtype=np.float32)
    Wv = np.asarray(Wv, dtype=np.float32)
    bv = np.asarray(bv, dtype=np.float32)
    E = np.asarray(E, dtype=np.float32)

    scale = 1.0 / math.sqrt(DH)

    def block_w(w):
        # [D, J] -> [P(pi), DC(po), J] with d = po*P + pi
        return np.ascontiguousarray(
            w.reshape(DC, P, J).transpose(1, 0, 2).astype(np_bf16)
        )

    # x[b].T is [D, L]; block to [P(pi), NLC, DC(po), LCH]
    xTs = [
        np.ascontiguousarray(
            x[b].T.reshape(DC, P, NLC, LCH).transpose(1, 2, 0, 3).astype(np_bf16)
        )
        for b in range(B)
    ]
    in_maps = []
    for core in range(NCORES):
        b = core % B
        hg = core // B
        js = slice(hg * J, (hg + 1) * J)
        hs = slice(hg * H, (hg + 1) * H)
        wqTs = block_w((Wq[js, :] * scale).T)
        wkTs = block_w(Wk[js, :].T)
        wvTs = block_w(Wv[js, :].T)
        bqTs = np.ascontiguousarray((bq[js] * scale).reshape(JT, P).T)
        bkBs = np.ascontiguousarray(np.broadcast_to(bk[js], (P, J)))
        bvBs = np.ascontiguousarray(np.broadcast_to(bv[js], (P, J)))
        E_s = E[hs]  # [H, KK, L]
        eTs = np.ascontiguousarray(
            E_s.reshape(H, KK, NLC, LT4, P).transpose(2, 4, 0, 3, 1).astype(np_bf16)
        )  # [NLC, P, H, LT4, KK]
        in_maps.append(
            {
                "xT": xTs[b],
                "wqT": wqTs,
                "wkT": wkTs,
                "wvT": wvTs,
                "bqT": bqTs,
                "bkB": bkBs,
                "bvB": bvBs,
                "eT": eTs,
            }
        )

    nc = _get_program()
    try:
        res = run_bass_kernel_spmd(nc, in_maps, list(range(NCORES)), trace=TRACE)
    except Exception:
        # rare transient NRT_EXEC_UNIT_UNRECOVERABLE flake observed on this
        # fleet (~2 in 25 runs, never twice in a row) — retry once
        res = run_bass_kernel_spmd(nc, in_maps, list(range(NCORES)), trace=TRACE)
    LAST_RESULTS = res

    outp = np.empty((B, L, D), dtype=np.float32)
    for core in range(NCORES):
        b = core % B
        hg = core // B
        # device layout [P(li), NLC, LT4, J] with l = lc*LCH + lt*P + li
        dev = res.results[core]["out"].astype(np.float32)
        outp[b, :, hg * J : (hg + 1) * J] = dev.transpose(1, 2, 0, 3).reshape(L, J)
    return outp


# revision 63
# speedup vs baseline: 1.0377x; 1.0377x over previous
"""Linformer attention TRN2 Bass kernel (bf16 matmul path).

Problem: nn_LinformerAttention (B=4, L=4096, D=1024, NH=16, DH=64, k=128).

Sharding: 8 cores = batch(4) x head-group(2). Core c handles batch c%4 and
heads (c//4)*8 .. +8, producing out[b, :, hg*512:(hg+1)*512]. Slices are
disjoint -> no collectives; host reassembles.

Device algorithm per core (matmul operands bf16, PSUM/accum fp32):
  phase 1, streamed over 8 l-chunks of 512:
    - K = x @ Wk.T + bk, V likewise (PSUM accum over 8 d-subtiles of 128),
      cast to bf16 in SBUF
    - Q.T = Wq @ x.T + bq (scaled by 1/sqrt(dh) folded into Wq/bq on host),
      kept RESIDENT in SBUF as bf16 (no DRAM spill)
    - KVp[h] += E_h.T-chunk.T @ [K_h | V_h]  (Linformer projection, both
      [k=128, dh=64], accumulated in fp32 SBUF via DVE adds)
  phase 2:
    - KpT pair tiles [128, k]: heads 2j/2j+1 at partitions 0:64/64:128
      (PE transpose); Vp_aug[h] = [Vp[h] | ones] in bf16
    - dotT[k, l] = KpT_h.T @ Q.T-chunk   (one matmul per (h, l-chunk))
    - expT = exp(dotT)  (ACT, bf16 out; logits small by construction)
    - Xo_aug = expT-tile.T @ Vp_aug -> [l-tile, 65]; col 64 = softmax denom
    - out[:, h*64:+64] = Xo_aug[:, :64] * 1/Xo_aug[:, 64]   (fp32)

Host prep (numpy, outside HW-timed region): x[b].T, W slices pre-transposed
(+1/8 scale on Wq), E head-slices pre-transposed, bias tiles; matmul
operands cast to bf16.
"""

import sys

sys.path.insert(0, "/opt/trn_rl_repo")

import math
from contextlib import ExitStack

import numpy as np
from ml_dtypes import bfloat16 as np_bf16

import json

import concourse.bass as bass
import concourse.bass2jax as bass2jax
import concourse.mybir as mybir
import concourse.tile as tile
from concourse.bass_utils import compile_bir_kernel as _orig_compile_bir_kernel
from concourse.bass_utils import run_bass_kernel_spmd
from concourse.masks import make_identity


def _split_multiwaits(bir_json_bytes):
    """This container's walrus encodes at most ONE sync wait per engine
    instruction ("Too many sync wait commands" otherwise), while Tile emits
    multi-wait instructions. Hoist extra waits onto single-wait
    EventSemaphore carrier instructions placed just before, on the same
    engine queue — semantically identical stalling."""
    bj = json.loads(bir_json_bytes)
    for fn in bj["functions"]:
        for blk in fn["blocks"]:
            out = []
            for inst in blk["instructions"]:
                si = inst.get("sync_info")
                waits = (si or {}).get("on_wait") or []
                if si and len(waits) > 1:
                    for wi, w in enumerate(waits[:-1]):
                        out.append(
                            {
                                "debug": inst.get("debug", 0),
                                "engine": inst.get("engine"),
                                "ins": [],
                                "outs": [],
                                "name": inst["name"] + "-w%d" % wi,
                                "opcode": "EventSemaphore",
                                "sync_info": {"on_update": [], "on_wait": [w]},
                            }
                        )
                    si["on_wait"] = [waits[-1]]
                out.append(inst)
            blk["instructions"] = out
    return json.dumps(bj).encode()


def _patched_compile_bir_kernel(bir_json, tmpdir, neff_name="file.neff"):
    return _orig_compile_bir_kernel(_split_multiwaits(bir_json), tmpdir, neff_name)


bass2jax.compile_bir_kernel = _patched_compile_bir_kernel

B, L, D = 4, 4096, 1024
NH, DH, KK = 16, 64, 128
NCORES = 8
HGS = 2  # head groups
H = NH // HGS  # 8 local heads per core
J = H * DH  # 512 output columns per core
P = 128
LCH = 512  # l-chunk
NLC = L // LCH  # 8
DC = D // P  # 8 contraction subtiles
JT = J // P  # 4
LT4 = LCH // P  # 4 l-tiles per chunk
F32 = mybir.dt.float32
BF16 = mybir.dt.bfloat16

TRACE = False  # test.py sets True to collect a profile
LAST_RESULTS = None  # BassKernelResults of the last kernel() call

_PROGRAM = None


def _build_program():
    nc = bass.Bass()
    # all tensors pre-blocked on host so every DMA descriptor is one
    # contiguous 8KB per-partition run (1KB descriptors cap a DMA queue at
    # ~100GB/s via descriptor-issue rate)
    xT = nc.declare_dram_parameter("xT", [P, NLC, DC, LCH], BF16, isOutput=False)
    wqT = nc.declare_dram_parameter("wqT", [P, DC, J], BF16, isOutput=False)
    wkT = nc.declare_dram_parameter("wkT", [P, DC, J], BF16, isOutput=False)
    wvT = nc.declare_dram_parameter("wvT", [P, DC, J], BF16, isOutput=False)
    bqT = nc.declare_dram_parameter("bqT", [P, JT], F32, isOutput=False)
    bkB = nc.declare_dram_parameter("bkB", [P, J], F32, isOutput=False)
    bvB = nc.declare_dram_parameter("bvB", [P, J], F32, isOutput=False)
    eT = nc.declare_dram_parameter("eT", [NLC, P, H, LT4, KK], BF16, isOutput=False)
    out = nc.declare_dram_parameter("out", [P, NLC, LT4, J], BF16, isOutput=True)

    add = mybir.AluOpType.add
    mult = mybir.AluOpType.mult

    with tile.TileContext(nc) as tc:
        with ExitStack() as ctx:
            const = ctx.enter_context(tc.tile_pool(name="const", bufs=1))
            xpool = ctx.enter_context(tc.tile_pool(name="x", bufs=2))
            kvpool = ctx.enter_context(tc.tile_pool(name="kv", bufs=4))
            epool = ctx.enter_context(tc.tile_pool(name="e", bufs=2))
            exppool = ctx.enter_context(tc.tile_pool(name="ex", bufs=3))
            outpool = ctx.enter_context(tc.tile_pool(name="ot", bufs=2))
            recpool = ctx.enter_context(tc.tile_pool(name="rc", bufs=8))


            # ---- constants resident in SBUF (K/V weights first: first matmuls
            # need only wk/wv + the first x chunk; wq is DMA'd after the first
            # chunk's x/e so it doesn't delay them in the queue)
            wq_sb = const.tile([P, DC, J], BF16, tag="wq")
            wk_sb = const.tile([P, DC, J], BF16, tag="wk")
            wv_sb = const.tile([P, DC, J], BF16, tag="wv")
            bqT_sb = const.tile([P, JT], F32, tag="bqT")
            bkB_sb = const.tile([P, J], F32, tag="bkB")
            bvB_sb = const.tile([P, J], F32, tag="bvB")
            ident = const.tile([P, P], F32, tag="ident")
            make_identity(nc, ident[:])

            # Q kept resident in SBUF, layout [j%128, lc, j//128, l%512]
            qt_all = const.tile([P, NLC, JT, LCH], BF16, tag="qt")

            # per head-pair accum: [kk, {K,V}, dh-of-head-2j | dh-of-head-2j+1]
            kvp2 = [const.tile([P, 2, 2 * DH], F32, tag=f"kvp{j}", name=f"kvp{j}") for j in range(JT)]
            # per head-pair: heads 2j, 2j+1 at partitions 0:64 / 64:128
            kpT2 = [const.tile([P, KK], BF16, tag=f"kpT{j}", name=f"kpT{j}") for j in range(JT)]
            vpa = [const.tile([P, DH + 1], BF16, tag=f"vpa{h}", name=f"vpa{h}") for h in range(H)]



            # ---- phase 1: projections + Linformer K/V reduction
            with ExitStack() as ctx1:
                psA = ctx1.enter_context(tc.tile_pool(name="psA", bufs=4, space="PSUM"))
                psB = ctx1.enter_context(tc.tile_pool(name="psB", bufs=4, space="PSUM"))

                # Warm-up: make PE observe each weight DMA individually, so no
                # later matmul ever needs two DMA-queue waits at once (the PE
                # Matmult encoding only fits one sync wait -> neuronxcc
                # "Too many sync wait commands" otherwise).
                def warm(wi, w_sb):
                    ps_w = psB.tile([1, 1], F32, tag="small", name=f"warm{wi}")
                    nc.tensor.matmul(
                        ps_w[:], w_sb[:, 0, 0:1],
                        w_sb[:, 0, 0:1],
                        start=True, stop=True,
                    )

                warm(0, wk_sb)
                for lc in range(NLC):
                    x_sb = xpool.tile([P, DC, LCH], BF16, tag="x")
                    e_sb = epool.tile([P, H, LT4, KK], BF16, tag="e")
                    if lc == 0:
                        # startup split: sync queue carries x0, wk; the second
                        # (Activation) queue carries wv, e0, biases, wq — so
                        # both K and V weights land in parallel
                        nc.sync.dma_start(x_sb[:], xT[:, lc])
                        nc.sync.dma_start(wk_sb[:], wkT[:, :, :])
                        nc.scalar.dma_start(wv_sb[:], wvT[:, :, :])
                        nc.scalar.dma_start(e_sb[:], eT[lc])
                        nc.scalar.dma_start(bkB_sb[:], bkB[:, :])
                        nc.scalar.dma_start(bvB_sb[:], bvB[:, :])
                        nc.scalar.dma_start(bqT_sb[:], bqT[:, :])
                        nc.scalar.dma_start(wq_sb[:], wqT[:, :, :])
                    else:
                        nc.sync.dma_start(x_sb[:], xT[:, lc])
                        nc.sync.dma_start(e_sb[:], eT[lc])
                    kv_tiles = []
                    for lt in range(LT4):
                        psK = psA.tile([P, LCH], F32, tag="big")
                        psV = psA.tile([P, LCH], F32, tag="big")
                        if lc == 0 and lt == 0:
                            # all-K first so compute starts before wv lands
                            for dc in range(DC):
                                nc.tensor.matmul(
                                    psK[:], x_sb[:, dc, lt * P : (lt + 1) * P],
                                    wk_sb[:, dc, :],
                                    start=(dc == 0), stop=(dc == DC - 1),
                                )
                            for dc in range(DC):
                                nc.tensor.matmul(
                                    psV[:], x_sb[:, dc, lt * P : (lt + 1) * P],
                                    wv_sb[:, dc, :],
                                    start=(dc == 0), stop=(dc == DC - 1),
                                )
                        else:
                            for dc in range(DC):
                                xst = x_sb[:, dc, lt * P : (lt + 1) * P]
                                nc.tensor.matmul(
                                    psK[:], xst,
                                    wk_sb[:, dc, :],
                                    start=(dc == 0), stop=(dc == DC - 1),
                                )
                                nc.tensor.matmul(
                                    psV[:], xst,
                                    wv_sb[:, dc, :],
                                    start=(dc == 0), stop=(dc == DC - 1),
                                )
                        kv_sb = kvpool.tile([P, 2, LCH], BF16, tag="kv")
                        nc.any.tensor_tensor(kv_sb[:, 0, :], psK[:], bkB_sb[:], add)
                        nc.any.tensor_tensor(kv_sb[:, 1, :], psV[:], bvB_sb[:], add)
                        kv_tiles.append(kv_sb)
                    for jt in range(JT):
                        psQ = psA.tile([P, LCH], F32, tag="big")
                        psC = psB.tile([P, 2, 2 * DH], F32, tag="small")
                        # Q (512-row) and Linformer psC (128-row) matmuls
                        # interleaved 1:1 so every small matmul's LDWEIGHTS
                        # prefetches under the preceding big matmul
                        for dc in range(DC):
                            nc.tensor.matmul(
                                psQ[:], wq_sb[:, dc, jt * P : (jt + 1) * P],
                                x_sb[:, dc, :],
                                start=(dc == 0), stop=(dc == DC - 1),
                            )
                            hp = dc // LT4  # head parity within the pair
                            h = 2 * jt + hp
                            lt = dc % LT4
                            nc.tensor.matmul(
                                psC[:, :, hp * DH : (hp + 1) * DH],
                                e_sb[:, h, lt, :],
                                kv_tiles[lt][:, :, h * DH : (h + 1) * DH],
                                start=(lt == 0), stop=(lt == LT4 - 1),
                            )
                        nc.any.tensor_scalar(
                            qt_all[:, lc, jt, :], psQ[:], bqT_sb[:, jt : jt + 1], None, add
                        )
                        if lc == 0:
                            nc.any.tensor_copy(kvp2[jt][:], psC[:])
                        else:
                            nc.any.tensor_tensor(kvp2[jt][:], kvp2[jt][:], psC[:], add)

                # ---- Kp transpose + Vp-aug prep (still under psA scope; uses psB)
                for j in range(JT):
                    psT = psB.tile([P, KK], F32, tag="small")
                    nc.tensor.transpose(psT[:], kvp2[j][:, 0, :], ident[:])
                    nc.any.tensor_copy(kpT2[j][:], psT[:])
                for h in range(H):
                    nc.any.tensor_copy(
                        vpa[h][:, 0:DH], kvp2[h // 2][:, 1, (h % 2) * DH : (h % 2 + 1) * DH]
                    )
                    nc.any.memset(vpa[h][:, DH : DH + 1], 1.0)

            # ---- phase 2: attention (psA/psB released; one wide PSUM pool
            # shared by the dot tiles and the psX tiles — 4 bufs x 2 banks)
            with ExitStack() as ctx2:
                psDp = ctx2.enter_context(tc.tile_pool(name="psD", bufs=2, space="PSUM"))
                psXp = ctx2.enter_context(tc.tile_pool(name="psX", bufs=4, space="PSUM"))

                def psx_group(h, ex2, hp, ot):
                    # all 4 l-tiles of this head into one wide PSUM tile, then
                    # a single strided reciprocal + a single wide multiply
                    psXw = psXp.tile([P, LT4, DH + 1], F32, tag="xw")
                    for lt in range(LT4):
                        nc.tensor.matmul(
                            psXw[:, lt, :], ex2[:, hp, lt * P : (lt + 1) * P], vpa[h][:],
                            start=True, stop=True,
                        )
                    rc = recpool.tile([P, LT4, 1], F32, tag="rc")
                    nc.vector.reciprocal(rc[:], psXw[:, :, DH : DH + 1])
                    nc.any.tensor_tensor(
                        ot[:, :, h * DH : (h + 1) * DH],
                        psXw[:, :, 0:DH],
                        rc[:].to_broadcast([P, LT4, DH]),
                        mult,
                    )

                # one flat stream of (lc, pair) steps with a 1-step stagger
                # carried across lc boundaries: pair s-1's psX smalls are
                # emitted after pair s's dots so LDWEIGHTS prefetch hides
                steps = [(lc, j) for lc in range(NLC) for j in range(JT)]
                prev = None
                ot_cur = None

                def flush_prev(prev):
                    plc, pj, pex, pot = prev
                    for hp in range(2):
                        psx_group(2 * pj + hp, pex, hp, pot)
                    if pj == JT - 1:
                        nc.sync.dma_start(out[:, plc], pot[:])

                for lc, j in steps:
                    if j == 0:
                        ot_cur = outpool.tile([P, LT4, J], BF16, tag="ot")
                    psD2 = psDp.tile([P, 2, LCH], F32, tag="d2")
                    for hp in range(2):
                        par = hp * DH
                        nc.tensor.matmul(
                            psD2[:, hp, :],
                            kpT2[j][par : par + DH, :],
                            qt_all[par : par + DH, lc, j, :],
                            start=True, stop=True,
                        )
                    ex2 = exppool.tile([P, 2, LCH], BF16, tag="ex")
                    nc.scalar.activation(
                        ex2[:], psD2[:], mybir.ActivationFunctionType.Exp
                    )
                    if prev is not None:
                        flush_prev(prev)
                    prev = (lc, j, ex2, ot_cur)
                flush_prev(prev)

    return nc


def _get_program():
    global _PROGRAM
    if _PROGRAM is None:
        _PROGRAM = _build_program()
    return _PROGRAM


def kernel(x, Wq, bq, Wk, bk, Wv, bv, E):
    global LAST_RESULTS
    x = np.ascontiguousarray(np.asarray(x, dtype=np.float32))
    Wq = np.asarray(Wq, dtype=np.float32)
    bq = np.asarray(bq, dtype=np.float32)
    Wk = np.asarray(Wk, dtype=np.float32)
    bk = np.asarray(bk, dtype=np.float32)
    Wv = np.asarray(Wv, dtype=np.float32)
    bv = np.asarray(bv, dtype=np.float32)
    E = np.asarray(E, dtype=np.float32)

    scale = 1.0 / math.sqrt(DH)

    def block_w(w):
        # [D, J] -> [P(pi), DC(po), J] with d = po*P + pi
        return np.ascontiguousarray(
            w.reshape(DC, P, J).transpose(1, 0, 2).astype(np_bf16)
        )

    # x[b].T is [D, L]; block to [P(pi), NLC, DC(po), LCH]
    xTs = [
        np.ascontiguousarray(
            x[b].T.reshape(DC, P, NLC, LCH).transpose(1, 2, 0, 3).astype(np_bf16)
        )
        for b in range(B)
    ]
    in_maps = []
    for core in range(NCORES):
        b = core % B
        hg = core // B
        js = slice(hg * J, (hg + 1) * J)
        hs = slice(hg * H, (hg + 1) * H)
        wqTs = block_w((Wq[js, :] * scale).T)
        wkTs = block_w(Wk[js, :].T)
        wvTs = block_w(Wv[js, :].T)
        bqTs = np.ascontiguousarray((bq[js] * scale).reshape(JT, P).T)
        bkBs = np.ascontiguousarray(np.broadcast_to(bk[js], (P, J)))
        bvBs = np.ascontiguousarray(np.broadcast_to(bv[js], (P, J)))
        E_s = E[hs]  # [H, KK, L]
        eTs = np.ascontiguousarray(
            E_s.reshape(H, KK, NLC, LT4, P).transpose(2, 4, 0, 3, 1).astype(np_bf16)
        )  # [NLC, P, H, LT4, KK]
        in_maps.append(
            {
                "xT": xTs[b],
                "wqT": wqTs,
                "wkT": wkTs,
                "wvT": wvTs,
                "bqT": bqTs,
                "bkB": bkBs,
                "bvB": bvBs,
                "eT": eTs,
            }
        )

    nc = _get_program()
    res = run_bass_kernel_spmd(nc, in_maps, list(range(NCORES)), trace=TRACE)
    LAST_RESULTS = res

    outp = np.empty((B, L, D), dtype=np.float32)
    for core in range(NCORES):
        b = core % B
        hg = core // B
        # device layout [P(li), NLC, LT4, J] with l = lc*LCH + lt*P + li
        dev = res.results[core]["out"].astype(np.float32)
        outp[b, :, hg * J : (hg + 1) * J] = dev.transpose(1, 2, 0, 3).reshape(L, J)
    return outp
